# revision 4
# baseline (speedup 1.0000x reference)
"""Grouped per-feature linear (NLinear) on 8 Trainium2 NeuronCores.

Math: out[b,f,o] = sum_i x[b,f,i] * w[f,i,o] + bias[f,o]
  x: [8192, 64, 128] f32, w: [64, 128, 128] f32, bias: [64, 128] f32

Strategy:
  - Batch-shard across the 8 cores (1024 rows each); weights replicated (4 MB).
  - Host-side layout transforms only (no host compute): x is repacked so the
    contraction dim I lands on SBUF partitions with fully-linear DMA, and the
    device emits out^T per feature ([o, b]) which the host transposes back.
  - Per feature f: matmul(psum[O,512], lhsT=W_f[I,O], rhs=xT_f[I,512]) in fp32.
    Bias is folded into the PSUM->SBUF copy as a per-partition scalar add
    (bias varies along O = partitions in the out^T layout), split across the
    Scalar and Vector engines.
"""

import numpy as np

import concourse.bacc as bacc
import concourse.bass as bass
import concourse.mybir as mybir
import concourse.tile as tile
from concourse.bass_utils import run_bass_kernel_spmd

N_CORES = 8
B, F, I, O = 8192, 64, 128, 128
B_SHARD = B // N_CORES  # 1024
F_BLK = 2  # features per DMA block -> 1 MiB transfers
N_FREE = 512  # matmul moving free dim (fp32 max, one PSUM bank)
FP32 = mybir.dt.float32

_cached = None


def _build_bass():
    nc = bacc.Bacc("TRN2", debug=False, num_devices=N_CORES)

    xt = nc.dram_tensor("xt", [I, F, B_SHARD], FP32, kind="ExternalInput")
    wt = nc.dram_tensor("wt", [I, F, O], FP32, kind="ExternalInput")
    bt = nc.dram_tensor("bt", [O, F], FP32, kind="ExternalInput")
    out = nc.dram_tensor("out", [O, F, B_SHARD], FP32, kind="ExternalOutput")

    n_halves = B_SHARD // N_FREE  # 2

    with tile.TileContext(nc) as tc:
        with (
            tc.tile_pool(name="const", bufs=1) as cpool,
            tc.tile_pool(name="xin", bufs=3) as xpool,
            tc.tile_pool(name="oout", bufs=3) as opool,
            tc.tile_pool(name="ps", bufs=8, space="PSUM") as pspool,
        ):
            w_sb = cpool.tile([I, F, O], FP32)  # 4 MiB, resident
            nc.sync.dma_start(w_sb[:], wt[:])
            b_sb = cpool.tile([O, F], FP32)
            nc.sync.dma_start(b_sb[:], bt[:])

            for fb in range(F // F_BLK):
                x_t = xpool.tile([I, F_BLK, B_SHARD], FP32)
                nc.sync.dma_start(x_t[:], xt[:, fb * F_BLK : (fb + 1) * F_BLK, :])
                o_t = opool.tile([O, F_BLK, B_SHARD], FP32)
                for fi in range(F_BLK):
                    f = fb * F_BLK + fi
                    for h in range(n_halves):
                        ps = pspool.tile([O, N_FREE], FP32)
                        nc.tensor.matmul(
                            ps[:],
                            w_sb[:, f, :],
                            x_t[:, fi, h * N_FREE : (h + 1) * N_FREE],
                            start=True,
                            stop=True,
                        )
                        dst = o_t[:, fi, h * N_FREE : (h + 1) * N_FREE]
                        # psum -> sbuf copy with per-partition bias add;
                        # alternate engines to split the copy load.
                        if h % 2 == 0:
                            nc.vector.tensor_scalar_add(dst, ps[:], b_sb[:, f : f + 1])
                        else:
                            nc.scalar.add(dst, ps[:], b_sb[:, f : f + 1])
                nc.sync.dma_start(out[:, fb * F_BLK : (fb + 1) * F_BLK, :], o_t[:])

    nc.compile()
    return nc


def kernel(x, weight, bias):
    global _cached
    if _cached is None:
        _cached = _build_bass()
    nc = _cached

    x = np.asarray(x, dtype=np.float32)
    weight = np.asarray(weight, dtype=np.float32)
    bias = np.asarray(bias, dtype=np.float32)

    # [c, b, f, i] -> [c, i, f, b]
    xt = np.ascontiguousarray(x.reshape(N_CORES, B_SHARD, F, I).transpose(0, 3, 2, 1))
    wt = np.ascontiguousarray(weight.transpose(1, 0, 2))  # [i, f, o]
    bt = np.ascontiguousarray(bias.T)  # [o, f]

    in_maps = [{"xt": xt[c], "wt": wt, "bt": bt} for c in range(N_CORES)]
    res = run_bass_kernel_spmd(nc, in_maps, list(range(N_CORES))).results

    outs = np.stack([np.asarray(r["out"]) for r in res])  # [c, o, f, b]
    return np.ascontiguousarray(outs.transpose(0, 3, 2, 1)).reshape(B, F, O)
